# revision 7
# baseline (speedup 1.0000x reference)
"""Trainium2 Bass kernel for JointIntegralRegressor (soft-argmax over 3D heatmaps).

reference math (per (n,j) volume V[d,h,w] of shape 64^3):
    p = softmax(V.flatten())
    x = sum(p * w)/W - 0.5 ; y = sum(p * h)/H - 0.5 ; z = sum(p * d)/D - 0.5

softmax is shift-invariant, and inputs are standard-normal, so with E = exp(V)
(no max subtraction; exp(randn) is comfortably inside fp32/bf16 range):
    x = (sum w*E)/(sum E)/64 - 0.5   etc.

Per-core layout: a volume is 1 MiB contiguous -> SBUF [128, 2048] where
    partition p: d = p>>1, hpar = p&1   (h = 32*hpar + (f>>6))
    free f:      j = f>>6 (h low bits), w = f&63
Sums computed as:
    stage 1 (TensorE): strip[m, f] = sum_p W1[p, m] * E[p, f] with
        W1 cols = [1, d(p), hpar(p), 0]  -> per-volume [4, 2048] PSUM strip
    stage 2 (VectorE): per-partition reduce over f of PSUM [128,2048]
        plain        -> S (row 4v+0), sum d*E (row 4v+1), sum hpar*E (row 4v+2)
        * wpat(f&63) -> sum w*E (row 4v+0)
        * jpat(f>>6) -> sum (h&31)*E (row 4v+0)
    host: x=(XE/S)/64-0.5, y=((32*PE+JE)/S)/64-0.5, z=(ZE/S)/64-0.5
"""

import sys

if "/opt/trn_rl_repo" not in sys.path:
    sys.path.insert(0, "/opt/trn_rl_repo")

from contextlib import ExitStack

import ml_dtypes
import numpy as np

import concourse.bass as bass
import concourse.tile as tile
from concourse import bacc, mybir
from concourse.bass_utils import run_bass_kernel_spmd

N, J, D, H, W = 16, 24, 64, 64, 64
VOLS = N * J  # 384
NCORES = 8
VPC = VOLS // NCORES  # 48 volumes per core
P = 128
F = 2048  # free elems per partition per volume (64^3 / 128)
R0, R1 = 32, 16  # volumes per PSUM round

_cache = {}


def _build():
    nc = bacc.Bacc("TRN2", target_bir_lowering=False, debug=False)
    heat = nc.dram_tensor(
        "heat", [VPC, P, F], mybir.dt.float32, kind="ExternalInput"
    ).ap()
    # block-diagonal stage-1 weights: volume v uses block [:, 128v:128(v+1)],
    # whose only nonzero columns are 4v..4v+3 = [ones, d(p), hpar(p), 0].
    # PE requires matmul outputs at base partition 0, so every volume writes
    # the full [128, N] output and lands its rows via its own weight columns;
    # volumes accumulate into one PSUM tile (zero cols add zero elsewhere).
    w1 = nc.dram_tensor(
        "w1", [P, 128 * R0], mybir.dt.bfloat16, kind="ExternalInput"
    ).ap()
    wpat = nc.dram_tensor("wpat", [P, F], mybir.dt.float32, kind="ExternalInput").ap()
    jpat = nc.dram_tensor("jpat", [P, F], mybir.dt.float32, kind="ExternalInput").ap()
    out = nc.dram_tensor("out", [2, P, 3], mybir.dt.float32, kind="ExternalOutput").ap()

    with tile.TileContext(nc) as tc, ExitStack() as ctx:
        const = ctx.enter_context(tc.tile_pool(name="const", bufs=1))
        raws = ctx.enter_context(tc.tile_pool(name="raw", bufs=4))
        es = ctx.enter_context(tc.tile_pool(name="e", bufs=4))
        psums = ctx.enter_context(
            tc.tile_pool(name="ps", bufs=2, space=bass.MemorySpace.PSUM)
        )
        scratch = ctx.enter_context(tc.tile_pool(name="scr", bufs=2))
        res = ctx.enter_context(tc.tile_pool(name="res", bufs=2))

        w1_t = const.tile([P, 128 * R0], mybir.dt.bfloat16)
        nc.sync.dma_start(w1_t[:], w1[:])
        wpat_t = const.tile([P, F], mybir.dt.float32)
        nc.sync.dma_start(wpat_t[:], wpat[:])
        jpat_t = const.tile([P, F], mybir.dt.float32)
        nc.sync.dma_start(jpat_t[:], jpat[:])

        for r, nvol in enumerate((R0, R1)):
            pr = psums.tile([P, F], mybir.dt.float32)
            for v in range(nvol):
                g = r * R0 + v
                raw = raws.tile([P, F], mybir.dt.float32)
                # split the 1 MiB volume load into 4 DMAs on disjoint
                # partition/port groups so several HW queues stay busy
                for c in range(4):
                    nc.sync.dma_start(
                        raw[32 * c : 32 * (c + 1), :],
                        heat[g, 32 * c : 32 * (c + 1), :],
                    )
                e = es.tile([P, F], mybir.dt.bfloat16)
                nc.scalar.activation(
                    e[:], raw[:], mybir.ActivationFunctionType.Exp
                )
                for b in range(4):
                    nc.tensor.matmul(
                        pr[:, 512 * b : 512 * (b + 1)],
                        w1_t[:, 128 * v : 128 * (v + 1)],
                        e[:, 512 * b : 512 * (b + 1)],
                        start=(v == 0),
                        stop=(v == nvol - 1),
                    )
            npart = 4 * nvol
            t = res.tile([P, 3], mybir.dt.float32)
            nc.vector.reduce_sum(
                t[0:npart, 0:1], pr[0:npart, :], axis=mybir.AxisListType.X
            )
            # NB: tensor_tensor_reduce with a PSUM operand hard-faults the
            # exec unit on real TRN2 (sim accepts it) — use mul + reduce.
            for col, pat in ((1, wpat_t), (2, jpat_t)):
                prod = scratch.tile([P, F], mybir.dt.float32, tag="prod")
                nc.vector.tensor_mul(
                    prod[0:npart, :], pr[0:npart, :], pat[0:npart, :]
                )
                nc.vector.reduce_sum(
                    t[0:npart, col : col + 1],
                    prod[0:npart, :],
                    axis=mybir.AxisListType.X,
                )
            nc.sync.dma_start(out[r, 0:npart, :], t[0:npart, :])

    nc.compile()
    return nc


def _host_inputs():
    p = np.arange(P)
    w1 = np.zeros((P, 128 * R0), dtype=np.float32)
    for v in range(R0):
        w1[:, 128 * v + 4 * v + 0] = 1.0
        w1[:, 128 * v + 4 * v + 1] = p >> 1
        w1[:, 128 * v + 4 * v + 2] = p & 1
    w1 = w1.astype(ml_dtypes.bfloat16)
    f = np.arange(F)
    wpat = np.broadcast_to((f % 64).astype(np.float32), (P, F)).copy()
    jpat = np.broadcast_to((f // 64).astype(np.float32), (P, F)).copy()
    return w1, wpat, jpat


def _decode(outs):
    """outs: list of 8 arrays [2, 128, 3] -> preds [16, 24, 3] f32."""
    o = np.stack(outs).astype(np.float64)  # [8, 2, 128, 3]
    r0 = o[:, 0].reshape(NCORES, R0, 4, 3)
    r1 = o[:, 1, : 4 * R1].reshape(NCORES, R1, 4, 3)
    a = np.concatenate([r0, r1], axis=1).reshape(VOLS, 4, 3)
    S = a[:, 0, 0]
    XE = a[:, 0, 1]
    JE = a[:, 0, 2]
    ZE = a[:, 1, 0]
    PEs = a[:, 2, 0]
    x = XE / S / W - 0.5
    y = (32.0 * PEs + JE) / S / H - 0.5
    z = ZE / S / D - 0.5
    return np.stack([x, y, z], axis=1).astype(np.float32).reshape(N, J, 3)


def kernel(heatmaps, **run_kwargs):
    heatmaps = np.ascontiguousarray(np.asarray(heatmaps, dtype=np.float32))
    assert heatmaps.shape == (N, J, D, H, W)
    if "nc" not in _cache:
        _cache["nc"] = _build()
    nc = _cache["nc"]
    heat = heatmaps.reshape(VOLS, P, F)
    w1, wpat, jpat = _host_inputs()
    in_maps = [
        {
            "heat": heat[c * VPC : (c + 1) * VPC],
            "w1": w1,
            "wpat": wpat,
            "jpat": jpat,
        }
        for c in range(NCORES)
    ]
    res = run_bass_kernel_spmd(
        nc, in_maps, core_ids=list(range(NCORES)), **run_kwargs
    )
    preds = _decode([r["out"] for r in res.results])
    if run_kwargs:
        _cache["last_results"] = res
    return preds


# revision 10
# speedup vs baseline: 1.4809x; 1.4809x over previous
"""Trainium2 Bass kernel for JointIntegralRegressor (soft-argmax over 3D heatmaps).

reference math (per (n,j) volume V[d,h,w] of shape 64^3):
    p = softmax(V.flatten())
    x = sum(p * w)/W - 0.5 ; y = sum(p * h)/H - 0.5 ; z = sum(p * d)/D - 0.5

softmax is shift-invariant, and inputs are standard-normal, so with E = exp(V)
(no max subtraction; exp(randn) is comfortably inside fp32/bf16 range):
    x = (sum w*E)/(sum E)/64 - 0.5   etc.

Per-core layout: a volume is 1 MiB contiguous -> SBUF [128, 2048] where
    partition p: d = p>>1, hpar = p&1   (h = 32*hpar + (f>>6))
    free f:      j = f>>6 (h low bits), w = f&63
Sums computed as:
    stage 1 (TensorE): strip[m, f] = sum_p W1[p, m] * E[p, f] with
        W1 cols = [1, d(p), hpar(p), 0]  -> per-volume [4, 2048] PSUM strip
    stage 2 (VectorE): per-partition reduce over f of PSUM [128,2048]
        plain        -> S (row 4v+0), sum d*E (row 4v+1), sum hpar*E (row 4v+2)
        * wpat(f&63) -> sum w*E (row 4v+0)
        * jpat(f>>6) -> sum (h&31)*E (row 4v+0)
    host: x=(XE/S)/64-0.5, y=((32*PE+JE)/S)/64-0.5, z=(ZE/S)/64-0.5
"""

import sys

if "/opt/trn_rl_repo" not in sys.path:
    sys.path.insert(0, "/opt/trn_rl_repo")

from contextlib import ExitStack

import ml_dtypes
import numpy as np

import concourse.bass as bass
import concourse.tile as tile
from concourse import bacc, mybir
from concourse.bass_utils import run_bass_kernel_spmd

N, J, D, H, W = 16, 24, 64, 64, 64
VOLS = N * J  # 384
NCORES = 8
VPC = VOLS // NCORES  # 48 volumes per core
P = 128
F = 2048  # free elems per partition per volume (64^3 / 128)
R0, R1 = 32, 16  # volumes per PSUM round

_cache = {}


def _build():
    nc = bacc.Bacc("TRN2", target_bir_lowering=False, debug=False)
    heat = nc.dram_tensor(
        "heat", [VPC, P, F], mybir.dt.float32, kind="ExternalInput"
    ).ap()
    # block-diagonal stage-1 weights: volume v uses block [:, 128v:128(v+1)],
    # whose only nonzero columns are 4v..4v+3 = [ones, d(p), hpar(p), 0].
    # PE requires matmul outputs at base partition 0, so every volume writes
    # the full [128, N] output and lands its rows via its own weight columns;
    # volumes accumulate into one PSUM tile (zero cols add zero elsewhere).
    w1 = nc.dram_tensor(
        "w1", [P, 128 * R0], mybir.dt.bfloat16, kind="ExternalInput"
    ).ap()
    wpat = nc.dram_tensor("wpat", [P, F], mybir.dt.float32, kind="ExternalInput").ap()
    jpat = nc.dram_tensor("jpat", [P, F], mybir.dt.float32, kind="ExternalInput").ap()
    out = nc.dram_tensor("out", [2, P, 3], mybir.dt.float32, kind="ExternalOutput").ap()

    with tile.TileContext(nc) as tc, ExitStack() as ctx:
        const = ctx.enter_context(tc.tile_pool(name="const", bufs=1))
        raws = ctx.enter_context(tc.tile_pool(name="raw", bufs=4))
        es = ctx.enter_context(tc.tile_pool(name="e", bufs=4))
        psums = ctx.enter_context(
            tc.tile_pool(name="ps", bufs=2, space=bass.MemorySpace.PSUM)
        )
        scratch = ctx.enter_context(tc.tile_pool(name="scr", bufs=2))
        res = ctx.enter_context(tc.tile_pool(name="res", bufs=2))

        w1_t = const.tile([P, 128 * R0], mybir.dt.bfloat16)
        nc.sync.dma_start(w1_t[:], w1[:])
        wpat_t = const.tile([P, F], mybir.dt.float32)
        nc.sync.dma_start(wpat_t[:], wpat[:])
        jpat_t = const.tile([P, F], mybir.dt.float32)
        nc.sync.dma_start(jpat_t[:], jpat[:])

        for r, nvol in enumerate((R0, R1)):
            pr = psums.tile([P, F], mybir.dt.float32)
            # volumes in pairs: one 2 MiB dma_start per pair (a single
            # dma_start is split across all 16 SDMA engines; >=1 MiB is
            # needed for full HBM bandwidth -- small transfers are
            # descriptor/fixed-cost dominated)
            for u in range(nvol // 2):
                g = r * R0 + 2 * u
                raw = raws.tile([P, 2 * F], mybir.dt.float32)
                nc.sync.dma_start(
                    raw[:].rearrange("p (v f) -> p v f", v=2),
                    heat[g : g + 2].rearrange("v p f -> p v f"),
                )
                e = es.tile([P, 2 * F], mybir.dt.bfloat16)
                nc.scalar.activation(
                    e[:], raw[:], mybir.ActivationFunctionType.Exp
                )
                for k in range(2):
                    v = 2 * u + k
                    for b in range(4):
                        nc.tensor.matmul(
                            pr[:, 512 * b : 512 * (b + 1)],
                            w1_t[:, 128 * v : 128 * (v + 1)],
                            e[:, k * F + 512 * b : k * F + 512 * (b + 1)],
                            start=(v == 0),
                            stop=(v == nvol - 1),
                        )
            npart = 4 * nvol
            t = res.tile([P, 3], mybir.dt.float32)
            nc.vector.reduce_sum(
                t[0:npart, 0:1], pr[0:npart, :], axis=mybir.AxisListType.X
            )
            # NB: tensor_tensor_reduce with a PSUM operand hard-faults the
            # exec unit on real TRN2 (sim accepts it) — use mul + reduce.
            for col, pat in ((1, wpat_t), (2, jpat_t)):
                prod = scratch.tile([P, F], mybir.dt.float32, tag="prod")
                nc.vector.tensor_mul(
                    prod[0:npart, :], pr[0:npart, :], pat[0:npart, :]
                )
                nc.vector.reduce_sum(
                    t[0:npart, col : col + 1],
                    prod[0:npart, :],
                    axis=mybir.AxisListType.X,
                )
            # tiny result store via SWDGE: HWDGE rings are FIFO per engine,
            # so putting this on nc.sync would block round r+1's input
            # loads behind the stage-2 DVE chain (~40us stall observed)
            nc.gpsimd.dma_start(out[r, 0:npart, :], t[0:npart, :])

    nc.compile()
    return nc


def _host_inputs():
    p = np.arange(P)
    w1 = np.zeros((P, 128 * R0), dtype=np.float32)
    for v in range(R0):
        w1[:, 128 * v + 4 * v + 0] = 1.0
        w1[:, 128 * v + 4 * v + 1] = p >> 1
        w1[:, 128 * v + 4 * v + 2] = p & 1
    w1 = w1.astype(ml_dtypes.bfloat16)
    f = np.arange(F)
    wpat = np.broadcast_to((f % 64).astype(np.float32), (P, F)).copy()
    jpat = np.broadcast_to((f // 64).astype(np.float32), (P, F)).copy()
    return w1, wpat, jpat


def _decode(outs):
    """outs: list of 8 arrays [2, 128, 3] -> preds [16, 24, 3] f32."""
    o = np.stack(outs).astype(np.float64)  # [8, 2, 128, 3]
    r0 = o[:, 0].reshape(NCORES, R0, 4, 3)
    r1 = o[:, 1, : 4 * R1].reshape(NCORES, R1, 4, 3)
    a = np.concatenate([r0, r1], axis=1).reshape(VOLS, 4, 3)
    S = a[:, 0, 0]
    XE = a[:, 0, 1]
    JE = a[:, 0, 2]
    ZE = a[:, 1, 0]
    PEs = a[:, 2, 0]
    x = XE / S / W - 0.5
    y = (32.0 * PEs + JE) / S / H - 0.5
    z = ZE / S / D - 0.5
    return np.stack([x, y, z], axis=1).astype(np.float32).reshape(N, J, 3)


def kernel(heatmaps, **run_kwargs):
    heatmaps = np.ascontiguousarray(np.asarray(heatmaps, dtype=np.float32))
    assert heatmaps.shape == (N, J, D, H, W)
    if "nc" not in _cache:
        _cache["nc"] = _build()
    nc = _cache["nc"]
    heat = heatmaps.reshape(VOLS, P, F)
    w1, wpat, jpat = _host_inputs()
    in_maps = [
        {
            "heat": heat[c * VPC : (c + 1) * VPC],
            "w1": w1,
            "wpat": wpat,
            "jpat": jpat,
        }
        for c in range(NCORES)
    ]
    res = run_bass_kernel_spmd(
        nc, in_maps, core_ids=list(range(NCORES)), **run_kwargs
    )
    preds = _decode([r["out"] for r in res.results])
    if run_kwargs:
        _cache["last_results"] = res
    return preds


# revision 16
# speedup vs baseline: 1.6726x; 1.1295x over previous
"""Trainium2 Bass kernel for JointIntegralRegressor (soft-argmax over 3D heatmaps).

reference math (per (n,j) volume V[d,h,w] of shape 64^3):
    p = softmax(V.flatten())
    x = sum(p * w)/W - 0.5 ; y = sum(p * h)/H - 0.5 ; z = sum(p * d)/D - 0.5

softmax is shift-invariant, and inputs are standard-normal, so with E = exp(V)
(no max subtraction; exp(randn) is comfortably inside fp32/bf16 range):
    x = (sum w*E)/(sum E)/64 - 0.5   etc.

Per-core layout: a volume is 1 MiB contiguous -> SBUF [128, 2048] where
    partition p: d = p>>1, hpar = p&1   (h = 32*hpar + (f>>6))
    free f:      j = f>>6 (h low bits), w = f&63
Sums computed as:
    stage 1 (TensorE): strip[m, f] = sum_p W1[p, m] * E[p, f] with
        W1 cols = [1, d(p), hpar(p), 0]  -> per-volume [4, 2048] PSUM strip
    stage 2 (VectorE): per-partition reduce over f of PSUM [128,2048]
        plain        -> S (row 4v+0), sum d*E (row 4v+1), sum hpar*E (row 4v+2)
        * wpat(f&63) -> sum w*E (row 4v+0)
        * jpat(f>>6) -> sum (h&31)*E (row 4v+0)
    host: x=(XE/S)/64-0.5, y=((32*PE+JE)/S)/64-0.5, z=(ZE/S)/64-0.5
"""

import sys

if "/opt/trn_rl_repo" not in sys.path:
    sys.path.insert(0, "/opt/trn_rl_repo")

from contextlib import ExitStack

import ml_dtypes
import numpy as np

import concourse.bass as bass
import concourse.tile as tile
from concourse import bacc, mybir
from concourse.bass_utils import run_bass_kernel_spmd

N, J, D, H, W = 16, 24, 64, 64, 64
VOLS = N * J  # 384
NCORES = 8
VPC = VOLS // NCORES  # 48 volumes per core
P = 128
F = 2048  # free elems per partition per volume (64^3 / 128)
R0, R1 = 32, 16  # volumes per PSUM round

_cache = {}


def _build():
    nc = bacc.Bacc("TRN2", target_bir_lowering=False, debug=False)
    heat = nc.dram_tensor(
        "heat", [VPC, P, F], mybir.dt.float32, kind="ExternalInput"
    ).ap()
    # block-diagonal stage-1 weights: volume v uses block [:, 128v:128(v+1)],
    # whose only nonzero columns are 4v..4v+3 = [ones, d(p), hpar(p), 0].
    # PE requires matmul outputs at base partition 0, so every volume writes
    # the full [128, N] output and lands its rows via its own weight columns;
    # volumes accumulate into one PSUM tile (zero cols add zero elsewhere).
    w1 = nc.dram_tensor(
        "w1", [P, 128 * R0], mybir.dt.bfloat16, kind="ExternalInput"
    ).ap()
    out = nc.dram_tensor("out", [2, P, 3], mybir.dt.float32, kind="ExternalOutput").ap()

    with tile.TileContext(nc) as tc, ExitStack() as ctx:
        const = ctx.enter_context(tc.tile_pool(name="const", bufs=1))
        raws = ctx.enter_context(tc.tile_pool(name="raw", bufs=3))
        es = ctx.enter_context(tc.tile_pool(name="e", bufs=3))
        psums = ctx.enter_context(
            tc.tile_pool(name="ps", bufs=2, space=bass.MemorySpace.PSUM)
        )
        scratch = ctx.enter_context(tc.tile_pool(name="scr", bufs=2))
        res = ctx.enter_context(tc.tile_pool(name="res", bufs=2))

        # w1 on the scalar HWDGE ring: keeps the sync ring exclusively for
        # heat loads (HWDGE is FIFO per issuing engine, so anything queued
        # ahead of the first heat pair delays the whole pipeline ramp)
        w1_t = const.tile([P, 128 * R0], mybir.dt.bfloat16)
        nc.scalar.dma_start(w1_t[:], w1[:])
        # index patterns generated on-device (gpsimd iota + DVE cast)
        # instead of 2 MiB of DMA: wpat[p,f] = f%64, jpat[p,f] = f//64
        wpat_t = const.tile([P, F], mybir.dt.float32)
        jpat_t = const.tile([P, F], mybir.dt.float32)
        for pat_t, pattern in (
            (wpat_t, [[0, F // 64], [1, 64]]),
            (jpat_t, [[1, F // 64], [0, 64]]),
        ):
            ipat = const.tile([P, F], mybir.dt.int32, tag="ipat")
            nc.gpsimd.iota(
                ipat[:].rearrange("p (a b) -> p a b", b=64),
                pattern=pattern,
                base=0,
                channel_multiplier=0,
            )
            nc.vector.tensor_copy(pat_t[:], ipat[:])

        for r, nvol in enumerate((R0, R1)):
            pr = psums.tile([P, F], mybir.dt.float32)
            # volumes in pairs: one 2 MiB dma_start per pair (a single
            # dma_start is split across all 16 SDMA engines; >=1 MiB is
            # needed for full HBM bandwidth -- small transfers are
            # descriptor/fixed-cost dominated)
            for u in range(nvol // 2):
                g = r * R0 + 2 * u
                raw = raws.tile([P, 2 * F], mybir.dt.float32)
                nc.sync.dma_start(
                    raw[:].rearrange("p (v f) -> p v f", v=2),
                    heat[g : g + 2].rearrange("v p f -> p v f"),
                )
                e = es.tile([P, 2 * F], mybir.dt.bfloat16)
                nc.scalar.activation(
                    e[:], raw[:], mybir.ActivationFunctionType.Exp
                )
                for k in range(2):
                    v = 2 * u + k
                    for b in range(4):
                        nc.tensor.matmul(
                            pr[:, 512 * b : 512 * (b + 1)],
                            w1_t[:, 128 * v : 128 * (v + 1)],
                            e[:, k * F + 512 * b : k * F + 512 * (b + 1)],
                            start=(v == 0),
                            stop=(v == nvol - 1),
                        )
            npart = 4 * nvol
            t = res.tile([P, 3], mybir.dt.float32)
            # plain reduce on ACT (activation Copy + accum_out), weighted
            # reduces as single fused DVE passes (scalar_tensor_tensor).
            # NB: tensor_tensor_reduce with a PSUM operand hard-faults the
            # exec unit on real TRN2 (sim accepts it); STT is fine.
            cp = scratch.tile([P, F], mybir.dt.float32, tag="actcp")
            nc.scalar.activation(
                cp[0:npart, :],
                pr[0:npart, :],
                mybir.ActivationFunctionType.Copy,
                accum_out=t[0:npart, 0:1],
            )
            for col, pat in ((1, wpat_t), (2, jpat_t)):
                prod = scratch.tile([P, F], mybir.dt.float32, tag="prod")
                nc.vector.scalar_tensor_tensor(
                    out=prod[0:npart, :],
                    in0=pr[0:npart, :],
                    scalar=1.0,
                    in1=pat[0:npart, :],
                    op0=mybir.AluOpType.mult,
                    op1=mybir.AluOpType.mult,
                    accum_out=t[0:npart, col : col + 1],
                )
            # tiny result store via SWDGE: HWDGE rings are FIFO per engine,
            # so putting this on nc.sync would block round r+1's input
            # loads behind the stage-2 DVE chain (~40us stall observed)
            nc.gpsimd.dma_start(out[r, 0:npart, :], t[0:npart, :])

    nc.compile()
    return nc


def _host_inputs():
    p = np.arange(P)
    w1 = np.zeros((P, 128 * R0), dtype=np.float32)
    for v in range(R0):
        w1[:, 128 * v + 4 * v + 0] = 1.0
        w1[:, 128 * v + 4 * v + 1] = p >> 1
        w1[:, 128 * v + 4 * v + 2] = p & 1
    w1 = w1.astype(ml_dtypes.bfloat16)
    return w1


def _decode(outs):
    """outs: list of 8 arrays [2, 128, 3] -> preds [16, 24, 3] f32."""
    o = np.stack(outs).astype(np.float64)  # [8, 2, 128, 3]
    r0 = o[:, 0].reshape(NCORES, R0, 4, 3)
    r1 = o[:, 1, : 4 * R1].reshape(NCORES, R1, 4, 3)
    a = np.concatenate([r0, r1], axis=1).reshape(VOLS, 4, 3)
    S = a[:, 0, 0]
    XE = a[:, 0, 1]
    JE = a[:, 0, 2]
    ZE = a[:, 1, 0]
    PEs = a[:, 2, 0]
    x = XE / S / W - 0.5
    y = (32.0 * PEs + JE) / S / H - 0.5
    z = ZE / S / D - 0.5
    return np.stack([x, y, z], axis=1).astype(np.float32).reshape(N, J, 3)


def kernel(heatmaps, **run_kwargs):
    heatmaps = np.ascontiguousarray(np.asarray(heatmaps, dtype=np.float32))
    assert heatmaps.shape == (N, J, D, H, W)
    if "nc" not in _cache:
        _cache["nc"] = _build()
    nc = _cache["nc"]
    heat = heatmaps.reshape(VOLS, P, F)
    w1 = _host_inputs()
    in_maps = [
        {"heat": heat[c * VPC : (c + 1) * VPC], "w1": w1}
        for c in range(NCORES)
    ]
    res = run_bass_kernel_spmd(
        nc, in_maps, core_ids=list(range(NCORES)), **run_kwargs
    )
    preds = _decode([r["out"] for r in res.results])
    if run_kwargs:
        _cache["last_results"] = res
    return preds


# revision 17
# speedup vs baseline: 1.8475x; 1.1046x over previous
"""Trainium2 Bass kernel for JointIntegralRegressor (soft-argmax over 3D heatmaps).

reference math (per (n,j) volume V[d,h,w] of shape 64^3):
    p = softmax(V.flatten())
    x = sum(p * w)/W - 0.5 ; y = sum(p * h)/H - 0.5 ; z = sum(p * d)/D - 0.5

softmax is shift-invariant, and inputs are standard-normal, so with E = exp(V)
(no max subtraction; exp(randn) is comfortably inside fp32/bf16 range):
    x = (sum w*E)/(sum E)/64 - 0.5   etc.

Per-core layout: a volume is 1 MiB contiguous -> SBUF [128, 2048] where
    partition p: d = p>>1, hpar = p&1   (h = 32*hpar + (f>>6))
    free f:      j = f>>6 (h low bits), w = f&63
Sums computed as:
    stage 1 (TensorE): strip[m, f] = sum_p W1[p, m] * E[p, f] with
        W1 cols = [1, d(p), hpar(p), 0]  -> per-volume [4, 2048] PSUM strip
    stage 2 (VectorE): per-partition reduce over f of PSUM [128,2048]
        plain        -> S (row 4v+0), sum d*E (row 4v+1), sum hpar*E (row 4v+2)
        * wpat(f&63) -> sum w*E (row 4v+0)
        * jpat(f>>6) -> sum (h&31)*E (row 4v+0)
    host: x=(XE/S)/64-0.5, y=((32*PE+JE)/S)/64-0.5, z=(ZE/S)/64-0.5
"""

import sys

if "/opt/trn_rl_repo" not in sys.path:
    sys.path.insert(0, "/opt/trn_rl_repo")

from contextlib import ExitStack

import ml_dtypes
import numpy as np

import concourse.bass as bass
import concourse.tile as tile
from concourse import bacc, mybir
from concourse.bass_utils import run_bass_kernel_spmd

N, J, D, H, W = 16, 24, 64, 64, 64
VOLS = N * J  # 384
NCORES = 8
VPC = VOLS // NCORES  # 48 volumes per core
P = 128
F = 2048  # free elems per partition per volume (64^3 / 128)
R0, R1 = 32, 16  # volumes per PSUM round

_cache = {}


def _build():
    nc = bacc.Bacc("TRN2", target_bir_lowering=False, debug=False)
    heat = nc.dram_tensor(
        "heat", [VPC, P, F], mybir.dt.float32, kind="ExternalInput"
    ).ap()
    # block-diagonal stage-1 weights: volume v uses block [:, 128v:128(v+1)],
    # whose only nonzero columns are 4v..4v+3 = [ones, d(p), hpar(p), 0].
    # PE requires matmul outputs at base partition 0, so every volume writes
    # the full [128, N] output and lands its rows via its own weight columns;
    # volumes accumulate into one PSUM tile (zero cols add zero elsewhere).
    w1 = nc.dram_tensor(
        "w1", [P, 128 * R0], mybir.dt.bfloat16, kind="ExternalInput"
    ).ap()
    out = nc.dram_tensor("out", [2, P, 3], mybir.dt.float32, kind="ExternalOutput").ap()

    with tile.TileContext(nc) as tc, ExitStack() as ctx:
        const = ctx.enter_context(tc.tile_pool(name="const", bufs=1))
        raws = ctx.enter_context(tc.tile_pool(name="raw", bufs=3))
        es = ctx.enter_context(tc.tile_pool(name="e", bufs=3))
        psums = ctx.enter_context(
            tc.tile_pool(name="ps", bufs=2, space=bass.MemorySpace.PSUM)
        )
        scratch = ctx.enter_context(tc.tile_pool(name="scr", bufs=2))
        res = ctx.enter_context(tc.tile_pool(name="res", bufs=2))

        # w1 on the scalar HWDGE ring: keeps the sync ring exclusively for
        # heat loads (HWDGE is FIFO per issuing engine, so anything queued
        # ahead of the first heat pair delays the whole pipeline ramp)
        w1_t = const.tile([P, 128 * R0], mybir.dt.bfloat16)
        nc.scalar.dma_start(w1_t[:], w1[:])
        # index patterns generated on-device (gpsimd iota + DVE cast)
        # instead of 2 MiB of DMA: wpat[p,f] = f%64, jpat[p,f] = f//64
        wpat_t = const.tile([P, F], mybir.dt.float32)
        jpat_t = const.tile([P, F], mybir.dt.float32)
        for pat_t, pattern in (
            (wpat_t, [[0, F // 64], [1, 64]]),
            (jpat_t, [[1, F // 64], [0, 64]]),
        ):
            ipat = const.tile([P, F], mybir.dt.int32, tag="ipat")
            nc.gpsimd.iota(
                ipat[:].rearrange("p (a b) -> p a b", b=64),
                pattern=pattern,
                base=0,
                channel_multiplier=0,
            )
            nc.vector.tensor_copy(pat_t[:], ipat[:])

        # volume load batches: 2 MiB pairs in the steady state (a single
        # dma_start is split across all 16 SDMA engines; >=1 MiB is needed
        # for full HBM bandwidth), but 1 MiB singles at the very start
        # (first exp starts ~1 MiB sooner) and very end (the last in-flight
        # loads complete in a burst, so smaller grains halve the trailing
        # exp backlog after the final DMA lands)
        batches = {
            0: [(0, 1), (1, 1), (2, 1), (3, 1)]
            + [(g, 2) for g in range(4, R0, 2)],
            1: [(g, 2) for g in range(R0, R0 + R1 - 4, 2)]
            + [(g, 1) for g in range(R0 + R1 - 4, R0 + R1)],
        }
        for r, nvol in enumerate((R0, R1)):
            pr = psums.tile([P, F], mybir.dt.float32)
            for g0, nv in batches[r]:
                raw = raws.tile([P, nv * F], mybir.dt.float32, tag="raw")
                if nv == 1:
                    nc.sync.dma_start(raw[:], heat[g0])
                else:
                    nc.sync.dma_start(
                        raw[:].rearrange("p (v f) -> p v f", v=nv),
                        heat[g0 : g0 + nv].rearrange("v p f -> p v f"),
                    )
                e = es.tile([P, nv * F], mybir.dt.bfloat16, tag="e")
                for k in range(nv):
                    v = g0 - r * R0 + k
                    nc.scalar.activation(
                        e[:, k * F : (k + 1) * F],
                        raw[:, k * F : (k + 1) * F],
                        mybir.ActivationFunctionType.Exp,
                    )
                    for b in range(4):
                        nc.tensor.matmul(
                            pr[:, 512 * b : 512 * (b + 1)],
                            w1_t[:, 128 * v : 128 * (v + 1)],
                            e[:, k * F + 512 * b : k * F + 512 * (b + 1)],
                            start=(v == 0),
                            stop=(v == nvol - 1),
                        )
            npart = 4 * nvol
            t = res.tile([P, 3], mybir.dt.float32)
            # plain reduce on ACT (activation Copy + accum_out), weighted
            # reduces as single fused DVE passes (scalar_tensor_tensor).
            # NB: tensor_tensor_reduce with a PSUM operand hard-faults the
            # exec unit on real TRN2 (sim accepts it); STT is fine.
            cp = scratch.tile([P, F], mybir.dt.float32, tag="actcp")
            nc.scalar.activation(
                cp[0:npart, :],
                pr[0:npart, :],
                mybir.ActivationFunctionType.Copy,
                accum_out=t[0:npart, 0:1],
            )
            for col, pat in ((1, wpat_t), (2, jpat_t)):
                prod = scratch.tile([P, F], mybir.dt.float32, tag="prod")
                nc.vector.scalar_tensor_tensor(
                    out=prod[0:npart, :],
                    in0=pr[0:npart, :],
                    scalar=1.0,
                    in1=pat[0:npart, :],
                    op0=mybir.AluOpType.mult,
                    op1=mybir.AluOpType.mult,
                    accum_out=t[0:npart, col : col + 1],
                )
            # tiny result store via SWDGE: HWDGE rings are FIFO per engine,
            # so putting this on nc.sync would block round r+1's input
            # loads behind the stage-2 DVE chain (~40us stall observed)
            nc.gpsimd.dma_start(out[r, 0:npart, :], t[0:npart, :])

    nc.compile()
    return nc


def _host_inputs():
    p = np.arange(P)
    w1 = np.zeros((P, 128 * R0), dtype=np.float32)
    for v in range(R0):
        w1[:, 128 * v + 4 * v + 0] = 1.0
        w1[:, 128 * v + 4 * v + 1] = p >> 1
        w1[:, 128 * v + 4 * v + 2] = p & 1
    w1 = w1.astype(ml_dtypes.bfloat16)
    return w1


def _decode(outs):
    """outs: list of 8 arrays [2, 128, 3] -> preds [16, 24, 3] f32."""
    o = np.stack(outs).astype(np.float64)  # [8, 2, 128, 3]
    r0 = o[:, 0].reshape(NCORES, R0, 4, 3)
    r1 = o[:, 1, : 4 * R1].reshape(NCORES, R1, 4, 3)
    a = np.concatenate([r0, r1], axis=1).reshape(VOLS, 4, 3)
    S = a[:, 0, 0]
    XE = a[:, 0, 1]
    JE = a[:, 0, 2]
    ZE = a[:, 1, 0]
    PEs = a[:, 2, 0]
    x = XE / S / W - 0.5
    y = (32.0 * PEs + JE) / S / H - 0.5
    z = ZE / S / D - 0.5
    return np.stack([x, y, z], axis=1).astype(np.float32).reshape(N, J, 3)


def kernel(heatmaps, **run_kwargs):
    heatmaps = np.ascontiguousarray(np.asarray(heatmaps, dtype=np.float32))
    assert heatmaps.shape == (N, J, D, H, W)
    if "nc" not in _cache:
        _cache["nc"] = _build()
    nc = _cache["nc"]
    heat = heatmaps.reshape(VOLS, P, F)
    w1 = _host_inputs()
    in_maps = [
        {"heat": heat[c * VPC : (c + 1) * VPC], "w1": w1}
        for c in range(NCORES)
    ]
    res = run_bass_kernel_spmd(
        nc, in_maps, core_ids=list(range(NCORES)), **run_kwargs
    )
    preds = _decode([r["out"] for r in res.results])
    if run_kwargs:
        _cache["last_results"] = res
    return preds
